# revision 18
# baseline (speedup 1.0000x reference)
"""Trainium2 Bass kernel for nn_BasicConvolutionBlock (sparse conv rulebook +
GroupNorm + LeakyReLU), sharded over 8 NeuronCores.

Strategy
--------
- Shard the 300000 output rows across 8 cores (37500 rows each). Every pair
  (k, p) is owned by the core that owns out_idx[k, p]. Weights replicated.
- Per core, pairs are processed grouped by kernel offset k:
    gather feats rows (indirect DMA) -> PE transpose -> matmul with W[k]
    (4-way row-group packing) -> PE transpose back -> Y rows -> DRAM scratch.
- The scatter-add is performed as an indirect *gather-accumulate*: on the host
  we sort each core's rows by pair count, bucket them into 128-row tiles, and
  build per-(tile-group, slot) index tables addressing the Y scratch. A DMA
  gather with compute_op=add accumulates slot l of every row directly into an
  SBUF accumulator. Padding slots use an out-of-bounds index which the DMA
  bounds check skips.
- GroupNorm stats: free-dim reduces + a ones-vector matmul for the partition
  reduction, then a 16-float AllReduce across the 8 cores; affine + LeakyReLU
  fused afterwards; host inverts the row permutation.
"""

import os
import sys

import ml_dtypes
import numpy as np

sys.path.insert(0, "/opt/trn_rl_repo")

import concourse.bacc as bacc
import concourse.bass as bass
import concourse.tile as tile
from concourse import mybir
from concourse.masks import make_identity

F32 = mybir.dt.float32
BF16 = mybir.dt.bfloat16
I32 = mybir.dt.int32

# Full-size problem config (hardcoded; see spec)
N_POINTS = 300000
N_PAIRS = 100000
K_OFFSETS = 27
C_IN = 32
C_OUT = 64
GROUPS = 8
EPS = 1e-5
NEG_SLOPE = 0.01
NCORES = 8


class Cfg:
    def __init__(self, n_points, n_pairs, k_offsets, ncores, xgi=None):
        self.N = n_points
        self.NPAIRS = n_pairs
        self.K = k_offsets
        self.NCORES = ncores
        self.R = n_points // ncores          # real rows per core
        self.J = 16                          # chunks per gather instruction
        self.CHUNK = 128 * self.J            # pairs per gather instruction
        self.T = -(-self.R // 128)           # tiles of 128 rows (ceil)
        self.G = -(-self.T // self.J)        # tile groups of J tiles
        self.RT = self.G * self.J * 128      # padded rows per core
        if xgi is None:
            # instructions per k-offset: max pairs per (core, k) + safety
            mean = n_pairs / ncores
            xgi = int(np.ceil((mean + 6.0 * np.sqrt(mean) + 64) / self.CHUNK))
        self.XGI = xgi
        self.NY = self.K * self.XGI * self.CHUNK   # Y scratch rows per core


def host_prep(cfg, feats, weight, gamma, beta, in_idx, out_idx):
    """Build per-core input arrays + global slot schedule."""
    K, R, CHUNK, XGI, J = cfg.K, cfg.R, cfg.CHUNK, cfg.XGI, cfg.J
    n = cfg.N

    ii = np.ascontiguousarray(in_idx, dtype=np.int64).ravel()
    oo = np.ascontiguousarray(out_idx, dtype=np.int64).ravel()
    kk = np.repeat(np.arange(K, dtype=np.int32), cfg.NPAIRS)

    feats_aug = np.zeros((n + 1, C_IN), dtype=np.float32)
    feats_aug[:n] = np.asarray(feats, dtype=np.float32)

    owner = oo // R

    per_core = []
    L_group = np.zeros((cfg.NCORES, cfg.G), dtype=np.int64)
    for c in range(cfg.NCORES):
        sel = np.nonzero(owner == c)[0]
        kk_c = kk[sel]
        ii_c = ii[sel].astype(np.int64)
        rr_c = (oo[sel] - c * R).astype(np.int64)

        # ---- k-grouping and Y positions ----
        ordk = np.argsort(kk_c, kind="stable")
        nk = np.bincount(kk_c, minlength=K)
        assert nk.max() <= XGI * CHUNK, f"XGI too small: {nk.max()}"
        offs = np.zeros(K + 1, dtype=np.int64)
        np.cumsum(nk, out=offs[1:])
        pos_in_k = np.arange(len(ordk), dtype=np.int64) - offs[kk_c[ordk]]
        ypos = np.empty(len(ordk), dtype=np.int64)
        ypos[ordk] = kk_c[ordk] * (XGI * CHUNK) + pos_in_k

        # xg_idx[k, i, p, j] = in_idx of pair with ypos k*XGI*CHUNK + i*CHUNK + j*128 + p
        xg = np.full((K, XGI * CHUNK), n, dtype=np.int64)  # dummy -> zero row
        ii_sorted = ii_c[ordk]
        for k in range(K):
            xg[k, : nk[k]] = ii_sorted[offs[k] : offs[k + 1]]
        xg_idx = (
            xg.reshape(K, XGI, J, 128).transpose(0, 1, 3, 2).astype(np.int32)
        )  # [K, XGI, 128, J]
        # pre-gathered features in processing order: [K*XGI, 128, J*C_IN]
        feats_seq = feats_aug[xg_idx.reshape(K * XGI, 128, J)].reshape(
            K * XGI, 128, J * C_IN
        ).astype(ml_dtypes.bfloat16)

        # ---- row ranking by pair count (slot padding efficiency) ----
        counts = np.bincount(rr_c, minlength=R)
        order_rows = np.argsort(-counts, kind="stable")       # rank -> orig row
        rank_of_row = np.empty(R, dtype=np.int64)
        rank_of_row[order_rows] = np.arange(R)
        counts_sorted = counts[order_rows]

        # per-tile-group max L
        for g in range(cfg.G):
            s = g * J * 128
            L_group[c, g] = counts_sorted[s] if s < R else 0

        # ---- slot assignment ----
        ranks = rank_of_row[rr_c]
        ords = np.argsort(ranks, kind="stable")
        csort = np.zeros(R + 1, dtype=np.int64)
        np.cumsum(counts_sorted, out=csort[1:])
        slot = np.arange(len(ords), dtype=np.int64) - csort[ranks[ords]]
        per_core.append(
            dict(
                xg_idx=xg_idx,
                feats_seq=feats_seq,
                rank=ranks[ords],
                slot=slot,
                ypos=ypos[ords],
                order_rows=order_rows,
            )
        )

    # ---- global slot schedule ----
    Lg = L_group.max(axis=0)  # [G]
    maxL = int(Lg.max()) if len(Lg) else 0
    sched = [(g, l) for l in range(maxL) for g in range(cfg.G) if l < Lg[g]]
    S = len(sched)
    sched_pos = np.full((cfg.G, maxL if maxL else 1), -1, dtype=np.int64)
    for s, (g, l) in enumerate(sched):
        sched_pos[g, l] = s

    in_maps = []
    for c in range(cfg.NCORES):
        pc = per_core[c]
        slot_idx = np.full((S, 128, J), cfg.NY, dtype=np.int64)  # NY = invalid
        rank, slot, ypos = pc["rank"], pc["slot"], pc["ypos"]
        g_arr = rank // (J * 128)
        t_arr = (rank % (J * 128)) // 128
        p_arr = rank % 128
        s_arr = sched_pos[g_arr, slot]
        assert (s_arr >= 0).all()
        slot_idx[s_arr, p_arr, t_arr] = ypos
        nch = K * XGI
        nca = nch // 2
        in_maps.append(
            {
                "feats_seq_a": pc["feats_seq"][:nca],
                "feats_seq_b": pc["feats_seq"][nca:],
                "wmat": np.asarray(weight, dtype=np.float32).astype(
                    ml_dtypes.bfloat16
                ),
                "gamma": np.asarray(gamma, dtype=np.float32).reshape(1, C_OUT),
                "beta": np.asarray(beta, dtype=np.float32).reshape(1, C_OUT),
                "slot_idx": slot_idx.astype(np.int32),
            }
        )

    meta = dict(sched=sched, order_rows=[pc["order_rows"] for pc in per_core])
    return in_maps, meta


def build_program(cfg, sched, n_total_points):
    K, XGI, J, CHUNK, G = cfg.K, cfg.XGI, cfg.J, cfg.CHUNK, cfg.G
    S = len(sched)
    NY = cfg.NY

    nc = bacc.Bacc(
        "TRN2", target_bir_lowering=False, debug=False, num_devices=cfg.NCORES
    )

    NCH = K * XGI
    NCA = NCH // 2
    fs_a = nc.dram_tensor(
        "feats_seq_a", [NCA, 128, J * C_IN], BF16, kind="ExternalInput"
    )
    fs_b = nc.dram_tensor(
        "feats_seq_b", [NCH - NCA, 128, J * C_IN], BF16, kind="ExternalInput"
    )
    wmat = nc.dram_tensor("wmat", [K, C_IN, C_OUT], BF16, kind="ExternalInput")
    gamma = nc.dram_tensor("gamma", [1, C_OUT], F32, kind="ExternalInput")
    beta = nc.dram_tensor("beta", [1, C_OUT], F32, kind="ExternalInput")
    sl = nc.dram_tensor("slot_idx", [S, 128, J], I32, kind="ExternalInput")
    outt = nc.dram_tensor("out", [cfg.RT, C_OUT], F32, kind="ExternalOutput")
    y_scr = nc.dram_tensor("y_scratch", [NY + 1, C_OUT], BF16)

    with tile.TileContext(nc) as tc:
        with (
            tc.tile_pool(name="singles", bufs=1) as singles,
            tc.tile_pool(name="idxp", bufs=4) as idxp,
            tc.tile_pool(name="stgp", bufs=3) as stgp,
            tc.tile_pool(name="xtp", bufs=3) as xtp,
            tc.tile_pool(name="ytp", bufs=3) as ytp,
            tc.tile_pool(name="ysbp", bufs=3) as ysbp,
            tc.tile_pool(name="nrmp", bufs=3) as nrmp,
            tc.tile_pool(name="statp", bufs=1) as statp,
            tc.tile_pool(name="ps_t", bufs=2, space="PSUM") as ps_t,
            tc.tile_pool(name="ps_mm", bufs=1, space="PSUM") as ps_mm,
            tc.tile_pool(name="ps_y", bufs=2, space="PSUM") as ps_y,
            tc.tile_pool(name="dram", bufs=1, space="DRAM") as dram,
        ):
            ident = singles.tile([128, 128], F32)
            make_identity(nc, ident[:])
            identb = singles.tile([128, 128], BF16)
            make_identity(nc, identb[:])
            w4 = singles.tile([128, K, C_OUT], BF16)
            wsrc = wmat[:, :, :].rearrange("k i o -> i k o")
            for jj in range(4):
                nc.sync.dma_start(
                    out=w4[32 * jj : 32 * jj + 32, :, :],
                    in_=wsrc,
                )
            acc = singles.tile([128, G * J, C_OUT], F32)
            nc.vector.memset(acc[:], 0.0)
            zrow = singles.tile([1, C_OUT], BF16)
            nc.vector.memset(zrow[:], 0.0)
            nc.sync.dma_start(out=y_scr[NY : NY + 1, :], in_=zrow[:])

            # ---------------- Phase 1: streamed feats + conv, Y to DRAM -------
            for k in range(K):
                for i in range(XGI):
                    stg = stgp.tile([128, J, C_IN], BF16)
                    ch = k * XGI + i
                    fsrc = fs_a[ch, :, :] if ch < NCA else fs_b[ch - NCA, :, :]
                    nc.sync.dma_start(
                        out=stg[:].rearrange("p j c -> p (j c)"),
                        in_=fsrc,
                    )
                    ysb = ysbp.tile([128, J, C_OUT], BF16)
                    for m in range(J // 4):
                        xt_ps = ps_t.tile([128, 128], BF16)
                        nc.tensor.transpose(
                            out=xt_ps[:],
                            in_=stg[:, 4 * m : 4 * m + 4, :].rearrange(
                                "p a b -> p (a b)"
                            ),
                            identity=identb[:],
                        )
                        xt = xtp.tile([128, 128], BF16)
                        nc.vector.tensor_copy(out=xt[:], in_=xt_ps[:])
                        yt_ps = ps_mm.tile([64, 4, 512], F32)
                        for jj in range(4):
                            nc.tensor.matmul(
                                out=yt_ps[:, jj, :128],
                                lhsT=w4[32 * jj : 32 * jj + 32, k, :],
                                rhs=xt[32 * jj : 32 * jj + 32, :],
                                start=True,
                                stop=True,
                                tile_position=(32 * jj, 0),
                            )
                        yt = ytp.tile([64, 4, 128], F32)
                        nc.vector.tensor_copy(out=yt[:], in_=yt_ps[:, :, :128])
                        for jj in range(4):
                            yr_ps = ps_y.tile([128, 64], F32)
                            nc.tensor.transpose(
                                out=yr_ps[:],
                                in_=yt[:, jj, :],
                                identity=ident[:64, :64],
                            )
                            nc.scalar.copy(
                                out=ysb[:, 4 * m + jj, :], in_=yr_ps[:]
                            )
                    base = (k * XGI + i) * CHUNK
                    nc.sync.dma_start(
                        out=y_scr[base : base + CHUNK, :].rearrange(
                            "(j p) c -> p j c", p=128
                        ),
                        in_=ysb[:],
                    )

            # ---------------- Phase 2: gather + DVE-accumulate into acc -------
            for s, (g, l) in enumerate(sched):
                it = idxp.tile([128, J], I32)
                nc.sync.dma_start(out=it[:], in_=sl[s, :, :])
                stg2 = ysbp.tile([128, J, C_OUT], BF16, tag="stg2")
                for tx in range(J):
                    nc.gpsimd.indirect_dma_start(
                        out=stg2[:, tx, :],
                        out_offset=None,
                        in_=y_scr[:, :],
                        in_offset=bass.IndirectOffsetOnAxis(
                            ap=it[:, tx : tx + 1], axis=0
                        ),
                    )
                nc.vector.tensor_tensor(
                    out=acc[:, g * J : (g + 1) * J, :],
                    in0=acc[:, g * J : (g + 1) * J, :],
                    in1=stg2[:],
                    op=mybir.AluOpType.add,
                )

            # ---------------- Phase 3: GroupNorm stats + AllReduce ------------
            cg = C_OUT // GROUPS
            sums16 = statp.tile([128, 16], F32)
            accv = acc[:].rearrange("p t (grp c) -> p grp t c", grp=GROUPS, c=cg)
            nc.vector.reduce_sum(
                out=sums16[:, 0:GROUPS], in_=accv, axis=mybir.AxisListType.XY
            )
            sqpart = statp.tile([128, G, GROUPS], F32)
            for g in range(G):
                sq = nrmp.tile([128, J * C_OUT], F32)
                blk = acc[:, g * J : (g + 1) * J, :].rearrange("p t c -> p (t c)")
                nc.vector.tensor_tensor(
                    out=sq[:], in0=blk, in1=blk, op=mybir.AluOpType.mult
                )
                nc.vector.reduce_sum(
                    out=sqpart[:, g, :],
                    in_=sq[:].rearrange("p (t grp c) -> p grp t c", grp=GROUPS, c=cg),
                    axis=mybir.AxisListType.XY,
                )
            nc.vector.reduce_sum(
                out=sums16[:, GROUPS:16],
                in_=sqpart[:].rearrange("p g grp -> p grp g"),
                axis=mybir.AxisListType.X,
            )
            ones = singles.tile([128, 1], F32)
            nc.vector.memset(ones[:], 1.0)
            st_ps = ps_y.tile([16, 1], F32, tag="yr_ps")
            nc.tensor.matmul(
                out=st_ps[:], lhsT=sums16[:], rhs=ones[:], start=True, stop=True
            )
            st_sb = statp.tile([16, 1], F32)
            nc.vector.tensor_copy(out=st_sb[:], in_=st_ps[:])
            bounce_in = dram.tile([16, 1], F32)
            bounce_out = dram.tile([16, 1], F32)
            nc.sync.dma_start(out=bounce_in[:], in_=st_sb[:])
            nc.gpsimd.collective_compute(
                "AllReduce",
                mybir.AluOpType.add,
                replica_groups=[list(range(cfg.NCORES))],
                ins=[bounce_in.opt()],
                outs=[bounce_out.opt()],
            )
            st16 = statp.tile([1, 16], F32)
            nc.sync.dma_start(out=st16[:], in_=bounce_out[:].rearrange("a b -> b a"))

            # mean/var -> per-channel affine A, B
            inv_cnt = 1.0 / (float(n_total_points) * cg)
            mean8 = statp.tile([1, GROUPS], F32)
            nc.vector.tensor_scalar_mul(mean8[:], st16[:, 0:GROUPS], inv_cnt)
            msq8 = statp.tile([1, GROUPS], F32)
            nc.vector.tensor_scalar_mul(msq8[:], st16[:, GROUPS:16], inv_cnt)
            var8 = statp.tile([1, GROUPS], F32)
            nc.vector.tensor_tensor(
                out=var8[:], in0=mean8[:], in1=mean8[:], op=mybir.AluOpType.mult
            )
            nc.vector.tensor_tensor(
                out=var8[:], in0=msq8[:], in1=var8[:], op=mybir.AluOpType.subtract
            )
            eps_t = statp.tile([1, 1], F32)
            nc.vector.memset(eps_t[:], EPS)
            sd8 = statp.tile([1, GROUPS], F32)
            nc.scalar.activation(
                out=sd8[:],
                in_=var8[:],
                func=mybir.ActivationFunctionType.Sqrt,
                bias=eps_t[:],
                scale=1.0,
            )
            rstd8 = statp.tile([1, GROUPS], F32)
            nc.vector.reciprocal(out=rstd8[:], in_=sd8[:])

            gam = statp.tile([1, C_OUT], F32)
            bet = statp.tile([1, C_OUT], F32)
            nc.sync.dma_start(out=gam[:], in_=gamma[:, :])
            nc.sync.dma_start(out=bet[:], in_=beta[:, :])
            rstd64 = statp.tile([1, GROUPS, cg], F32)
            nc.vector.tensor_copy(
                out=rstd64[:],
                in_=rstd8[:].rearrange("p g -> p g ()").to_broadcast([1, GROUPS, cg]),
            )
            mean64 = statp.tile([1, GROUPS, cg], F32)
            nc.vector.tensor_copy(
                out=mean64[:],
                in_=mean8[:].rearrange("p g -> p g ()").to_broadcast([1, GROUPS, cg]),
            )
            a1 = statp.tile([1, C_OUT], F32)
            nc.vector.tensor_tensor(
                out=a1[:],
                in0=rstd64[:].rearrange("p g c -> p (g c)"),
                in1=gam[:],
                op=mybir.AluOpType.mult,
            )
            b1 = statp.tile([1, C_OUT], F32)
            nc.vector.tensor_tensor(
                out=b1[:],
                in0=mean64[:].rearrange("p g c -> p (g c)"),
                in1=a1[:],
                op=mybir.AluOpType.mult,
            )
            nc.vector.tensor_tensor(
                out=b1[:], in0=bet[:], in1=b1[:], op=mybir.AluOpType.subtract
            )
            ab_dram = dram.tile([2, C_OUT], F32)
            nc.sync.dma_start(out=ab_dram[0:1, :], in_=a1[:])
            nc.sync.dma_start(out=ab_dram[1:2, :], in_=b1[:])
            a128 = singles.tile([128, C_OUT], F32)
            b128 = singles.tile([128, C_OUT], F32)
            nc.sync.dma_start(out=a128[:], in_=ab_dram[0:1, :].partition_broadcast(128))
            nc.sync.dma_start(out=b128[:], in_=ab_dram[1:2, :].partition_broadcast(128))

            # ---------------- Phase 4: normalize + LeakyReLU + store ----------
            for g in range(G):
                blk = acc[:, g * J : (g + 1) * J, :]
                t0 = nrmp.tile([128, J, C_OUT], F32)
                nc.vector.tensor_tensor(
                    out=t0[:],
                    in0=blk,
                    in1=a128[:].rearrange("p c -> p () c").to_broadcast(
                        [128, J, C_OUT]
                    ),
                    op=mybir.AluOpType.mult,
                )
                nc.vector.tensor_tensor(
                    out=t0[:],
                    in0=t0[:],
                    in1=b128[:].rearrange("p c -> p () c").to_broadcast(
                        [128, J, C_OUT]
                    ),
                    op=mybir.AluOpType.add,
                )
                t1 = nrmp.tile([128, J, C_OUT], F32)
                nc.vector.tensor_scalar_mul(t1[:], t0[:], NEG_SLOPE)
                nc.vector.tensor_tensor(
                    out=t0[:], in0=t0[:], in1=t1[:], op=mybir.AluOpType.max
                )
                nc.sync.dma_start(
                    out=outt[g * J * 128 : (g + 1) * J * 128, :].rearrange(
                        "(t p) c -> p t c", p=128
                    ),
                    in_=t0[:],
                )

    nc.compile()
    return nc


def _run(cfg, inputs, trace=False):
    from concourse import bass_utils

    in_maps, meta = host_prep(
        cfg,
        inputs["feats"],
        inputs["weight"],
        inputs["gamma"],
        inputs["beta"],
        inputs["in_idx"],
        inputs["out_idx"],
    )
    nc = build_program(cfg, meta["sched"], cfg.N)
    res = bass_utils.run_bass_kernel_spmd(
        nc, in_maps, core_ids=list(range(cfg.NCORES)), trace=trace
    )
    out = np.zeros((cfg.N, C_OUT), dtype=np.float32)
    for c in range(cfg.NCORES):
        oc = res.results[c]["out"]
        order = meta["order_rows"][c]
        out[c * cfg.R + order] = oc[: cfg.R]
    return out, res


def kernel(**inputs) -> np.ndarray:
    cfg = Cfg(N_POINTS, N_PAIRS, K_OFFSETS, NCORES)
    out, _ = _run(cfg, inputs, trace=False)
    return out



# revision 21
# speedup vs baseline: 1.1778x; 1.1778x over previous
"""Trainium2 Bass kernel for nn_BasicConvolutionBlock (sparse conv rulebook +
GroupNorm + LeakyReLU), sharded over 8 NeuronCores.

Strategy
--------
- Shard the 300000 output rows across 8 cores (37500 rows each). Every pair
  (k, p) is owned by the core that owns out_idx[k, p]. Weights replicated.
- Per core, pairs are processed grouped by kernel offset k:
    gather feats rows (indirect DMA) -> PE transpose -> matmul with W[k]
    (4-way row-group packing) -> PE transpose back -> Y rows -> DRAM scratch.
- The scatter-add is performed as an indirect *gather-accumulate*: on the host
  we sort each core's rows by pair count, bucket them into 128-row tiles, and
  build per-(tile-group, slot) index tables addressing the Y scratch. A DMA
  gather with compute_op=add accumulates slot l of every row directly into an
  SBUF accumulator. Padding slots use an out-of-bounds index which the DMA
  bounds check skips.
- GroupNorm stats: free-dim reduces + a ones-vector matmul for the partition
  reduction, then a 16-float AllReduce across the 8 cores; affine + LeakyReLU
  fused afterwards; host inverts the row permutation.
"""

import os
import sys

import ml_dtypes
import numpy as np

sys.path.insert(0, "/opt/trn_rl_repo")

import concourse.bacc as bacc
import concourse.bass as bass
import concourse.tile as tile
from concourse import mybir
from concourse.masks import make_identity

F32 = mybir.dt.float32
BF16 = mybir.dt.bfloat16
I32 = mybir.dt.int32

# Full-size problem config (hardcoded; see spec)
N_POINTS = 300000
N_PAIRS = 100000
K_OFFSETS = 27
C_IN = 32
C_OUT = 64
GROUPS = 8
EPS = 1e-5
NEG_SLOPE = 0.01
NCORES = 8


class Cfg:
    def __init__(self, n_points, n_pairs, k_offsets, ncores, xgi=None):
        self.N = n_points
        self.NPAIRS = n_pairs
        self.K = k_offsets
        self.NCORES = ncores
        self.R = n_points // ncores          # real rows per core
        self.J = 16                          # chunks per gather instruction
        self.CHUNK = 128 * self.J            # pairs per gather instruction
        self.T = -(-self.R // 128)           # tiles of 128 rows (ceil)
        self.G = -(-self.T // self.J)        # tile groups of J tiles
        self.RT = self.G * self.J * 128      # padded rows per core
        if xgi is None:
            # instructions per k-offset: max pairs per (core, k) + safety
            mean = n_pairs / ncores
            xgi = int(np.ceil((mean + 6.0 * np.sqrt(mean) + 64) / self.CHUNK))
        self.XGI = xgi
        self.NY = self.K * self.XGI * self.CHUNK   # Y scratch rows per core


def host_prep(cfg, feats, weight, gamma, beta, in_idx, out_idx):
    """Build per-core input arrays + global slot schedule."""
    K, R, CHUNK, XGI, J = cfg.K, cfg.R, cfg.CHUNK, cfg.XGI, cfg.J
    n = cfg.N

    ii = np.ascontiguousarray(in_idx, dtype=np.int64).ravel()
    oo = np.ascontiguousarray(out_idx, dtype=np.int64).ravel()
    kk = np.repeat(np.arange(K, dtype=np.int32), cfg.NPAIRS)

    feats_aug = np.zeros((n + 1, C_IN), dtype=np.float32)
    feats_aug[:n] = np.asarray(feats, dtype=np.float32)

    owner = oo // R

    per_core = []
    L_group = np.zeros((cfg.NCORES, cfg.G), dtype=np.int64)
    for c in range(cfg.NCORES):
        sel = np.nonzero(owner == c)[0]
        kk_c = kk[sel]
        ii_c = ii[sel].astype(np.int64)
        rr_c = (oo[sel] - c * R).astype(np.int64)

        # ---- k-grouping and Y positions ----
        ordk = np.argsort(kk_c, kind="stable")
        nk = np.bincount(kk_c, minlength=K)
        assert nk.max() <= XGI * CHUNK, f"XGI too small: {nk.max()}"
        offs = np.zeros(K + 1, dtype=np.int64)
        np.cumsum(nk, out=offs[1:])
        pos_in_k = np.arange(len(ordk), dtype=np.int64) - offs[kk_c[ordk]]
        ypos = np.empty(len(ordk), dtype=np.int64)
        ypos[ordk] = kk_c[ordk] * (XGI * CHUNK) + pos_in_k

        # xg_idx[k, i, p, j] = in_idx of pair with ypos k*XGI*CHUNK + i*CHUNK + j*128 + p
        xg = np.full((K, XGI * CHUNK), n, dtype=np.int64)  # dummy -> zero row
        ii_sorted = ii_c[ordk]
        for k in range(K):
            xg[k, : nk[k]] = ii_sorted[offs[k] : offs[k + 1]]
        xg_idx = (
            xg.reshape(K, XGI, J, 128).transpose(0, 1, 3, 2).astype(np.int32)
        )  # [K, XGI, 128, J]
        # pre-gathered features in processing order: [K*XGI, 128, J*C_IN]
        feats_seq = feats_aug[xg_idx.reshape(K * XGI, 128, J)].reshape(
            K * XGI, 128, J * C_IN
        ).astype(ml_dtypes.bfloat16)

        # ---- row ranking by pair count (slot padding efficiency) ----
        counts = np.bincount(rr_c, minlength=R)
        order_rows = np.argsort(-counts, kind="stable")       # rank -> orig row
        rank_of_row = np.empty(R, dtype=np.int64)
        rank_of_row[order_rows] = np.arange(R)
        counts_sorted = counts[order_rows]

        # per-tile-group max L
        for g in range(cfg.G):
            s = g * J * 128
            L_group[c, g] = counts_sorted[s] if s < R else 0

        # ---- slot assignment ----
        ranks = rank_of_row[rr_c]
        ords = np.argsort(ranks, kind="stable")
        csort = np.zeros(R + 1, dtype=np.int64)
        np.cumsum(counts_sorted, out=csort[1:])
        slot = np.arange(len(ords), dtype=np.int64) - csort[ranks[ords]]
        per_core.append(
            dict(
                xg_idx=xg_idx,
                feats_seq=feats_seq,
                rank=ranks[ords],
                slot=slot,
                ypos=ypos[ords],
                order_rows=order_rows,
            )
        )

    # ---- global slot schedule ----
    Lg = L_group.max(axis=0)  # [G]
    maxL = int(Lg.max()) if len(Lg) else 0
    sched = [(g, l) for l in range(maxL) for g in range(cfg.G) if l < Lg[g]]
    S = len(sched)
    sched_pos = np.full((cfg.G, maxL if maxL else 1), -1, dtype=np.int64)
    for s, (g, l) in enumerate(sched):
        sched_pos[g, l] = s

    in_maps = []
    for c in range(cfg.NCORES):
        pc = per_core[c]
        slot_idx = np.full((S, 128, J), cfg.NY, dtype=np.int64)  # NY = invalid
        rank, slot, ypos = pc["rank"], pc["slot"], pc["ypos"]
        g_arr = rank // (J * 128)
        t_arr = (rank % (J * 128)) // 128
        p_arr = rank % 128
        s_arr = sched_pos[g_arr, slot]
        assert (s_arr >= 0).all()
        slot_idx[s_arr, p_arr, t_arr] = ypos
        nch = K * XGI
        nca = nch // 2
        in_maps.append(
            {
                "feats_seq_a": pc["feats_seq"][:nca],
                "feats_seq_b": pc["feats_seq"][nca:],
                "wmat": np.asarray(weight, dtype=np.float32).astype(
                    ml_dtypes.bfloat16
                ),
                "gamma": np.asarray(gamma, dtype=np.float32).reshape(1, C_OUT),
                "beta": np.asarray(beta, dtype=np.float32).reshape(1, C_OUT),
                "slot_idx": slot_idx.astype(np.int32),
            }
        )

    meta = dict(sched=sched, order_rows=[pc["order_rows"] for pc in per_core])
    return in_maps, meta


def build_program(cfg, sched, n_total_points):
    K, XGI, J, CHUNK, G = cfg.K, cfg.XGI, cfg.J, cfg.CHUNK, cfg.G
    S = len(sched)
    NY = cfg.NY

    nc = bacc.Bacc(
        "TRN2", target_bir_lowering=False, debug=False, num_devices=cfg.NCORES
    )

    NCH = K * XGI
    NCA = NCH // 2
    fs_a = nc.dram_tensor(
        "feats_seq_a", [NCA, 128, J * C_IN], BF16, kind="ExternalInput"
    )
    fs_b = nc.dram_tensor(
        "feats_seq_b", [NCH - NCA, 128, J * C_IN], BF16, kind="ExternalInput"
    )
    wmat = nc.dram_tensor("wmat", [K, C_IN, C_OUT], BF16, kind="ExternalInput")
    gamma = nc.dram_tensor("gamma", [1, C_OUT], F32, kind="ExternalInput")
    beta = nc.dram_tensor("beta", [1, C_OUT], F32, kind="ExternalInput")
    sl = nc.dram_tensor("slot_idx", [S, 128, J], I32, kind="ExternalInput")
    outt = nc.dram_tensor("out", [cfg.RT, C_OUT], F32, kind="ExternalOutput")
    y_scr = nc.dram_tensor("y_scratch", [NY + 1, C_OUT], F32)

    with tile.TileContext(nc) as tc:
        with (
            tc.tile_pool(name="singles", bufs=1) as singles,
            tc.tile_pool(name="idxp", bufs=4) as idxp,
            tc.tile_pool(name="stgp", bufs=3) as stgp,
            tc.tile_pool(name="xtp", bufs=3) as xtp,
            tc.tile_pool(name="ytp", bufs=3) as ytp,
            tc.tile_pool(name="ysbp", bufs=3) as ysbp,
            tc.tile_pool(name="nrmp", bufs=3) as nrmp,
            tc.tile_pool(name="statp", bufs=1) as statp,
            tc.tile_pool(name="ps_t", bufs=2, space="PSUM") as ps_t,
            tc.tile_pool(name="ps_mm", bufs=1, space="PSUM") as ps_mm,
            tc.tile_pool(name="ps_y", bufs=2, space="PSUM") as ps_y,
            tc.tile_pool(name="dram", bufs=1, space="DRAM") as dram,
        ):
            ident = singles.tile([128, 128], F32)
            make_identity(nc, ident[:])
            identb = singles.tile([128, 128], BF16)
            make_identity(nc, identb[:])
            w4 = singles.tile([128, K, C_OUT], BF16)
            wsrc = wmat[:, :, :].rearrange("k i o -> i k o")
            for jj in range(4):
                nc.sync.dma_start(
                    out=w4[32 * jj : 32 * jj + 32, :, :],
                    in_=wsrc,
                )
            acc = singles.tile([128, G * J, C_OUT], F32)
            nc.vector.memset(acc[:], 0.0)
            zrow = singles.tile([1, C_OUT], F32)
            nc.vector.memset(zrow[:], 0.0)
            nc.sync.dma_start(out=y_scr[NY : NY + 1, :], in_=zrow[:])

            # ---------------- Phase 1: streamed feats + conv, Y to DRAM -------
            for k in range(K):
                for i in range(XGI):
                    stg = stgp.tile([128, J, C_IN], BF16)
                    ch = k * XGI + i
                    fsrc = fs_a[ch, :, :] if ch < NCA else fs_b[ch - NCA, :, :]
                    nc.sync.dma_start(
                        out=stg[:].rearrange("p j c -> p (j c)"),
                        in_=fsrc,
                    )
                    ysb = ysbp.tile([128, J, C_OUT], F32)
                    for m in range(J // 4):
                        xt_ps = ps_t.tile([128, 128], BF16)
                        nc.tensor.transpose(
                            out=xt_ps[:],
                            in_=stg[:, 4 * m : 4 * m + 4, :].rearrange(
                                "p a b -> p (a b)"
                            ),
                            identity=identb[:],
                        )
                        xt = xtp.tile([128, 128], BF16)
                        nc.vector.tensor_copy(out=xt[:], in_=xt_ps[:])
                        yt_ps = ps_mm.tile([64, 4, 512], F32)
                        for jj in range(4):
                            nc.tensor.matmul(
                                out=yt_ps[:, jj, :128],
                                lhsT=w4[32 * jj : 32 * jj + 32, k, :],
                                rhs=xt[32 * jj : 32 * jj + 32, :],
                                start=True,
                                stop=True,
                                tile_position=(32 * jj, 0),
                            )
                        yt = ytp.tile([64, 4, 128], F32)
                        nc.vector.tensor_copy(out=yt[:], in_=yt_ps[:, :, :128])
                        for jj in range(4):
                            yr_ps = ps_y.tile([128, 64], F32)
                            nc.tensor.transpose(
                                out=yr_ps[:],
                                in_=yt[:, jj, :],
                                identity=ident[:64, :64],
                            )
                            nc.scalar.copy(
                                out=ysb[:, 4 * m + jj, :], in_=yr_ps[:]
                            )
                    base = (k * XGI + i) * CHUNK
                    nc.sync.dma_start(
                        out=y_scr[base : base + CHUNK, :].rearrange(
                            "(j p) c -> p j c", p=128
                        ),
                        in_=ysb[:],
                    )

            # ---------------- Phase 2: gather + DVE-accumulate into acc -------
            for s, (g, l) in enumerate(sched):
                it = idxp.tile([128, J], I32)
                nc.sync.dma_start(out=it[:], in_=sl[s, :, :])
                stg2 = ysbp.tile([128, J, C_OUT], F32, tag="stg2")
                for tx in range(J):
                    nc.gpsimd.indirect_dma_start(
                        out=stg2[:, tx, :],
                        out_offset=None,
                        in_=y_scr[:, :],
                        in_offset=bass.IndirectOffsetOnAxis(
                            ap=it[:, tx : tx + 1], axis=0
                        ),
                    )
                nc.vector.tensor_tensor(
                    out=acc[:, g * J : (g + 1) * J, :],
                    in0=acc[:, g * J : (g + 1) * J, :],
                    in1=stg2[:],
                    op=mybir.AluOpType.add,
                )

            # ---------------- Phase 3: GroupNorm stats + AllReduce ------------
            cg = C_OUT // GROUPS
            sums16 = statp.tile([128, 16], F32)
            accv = acc[:].rearrange("p t (grp c) -> p grp t c", grp=GROUPS, c=cg)
            nc.vector.reduce_sum(
                out=sums16[:, 0:GROUPS], in_=accv, axis=mybir.AxisListType.XY
            )
            sqpart = statp.tile([128, G, GROUPS], F32)
            for g in range(G):
                sq = nrmp.tile([128, J * C_OUT], F32)
                blk = acc[:, g * J : (g + 1) * J, :].rearrange("p t c -> p (t c)")
                nc.vector.tensor_tensor(
                    out=sq[:], in0=blk, in1=blk, op=mybir.AluOpType.mult
                )
                nc.vector.reduce_sum(
                    out=sqpart[:, g, :],
                    in_=sq[:].rearrange("p (t grp c) -> p grp t c", grp=GROUPS, c=cg),
                    axis=mybir.AxisListType.XY,
                )
            nc.vector.reduce_sum(
                out=sums16[:, GROUPS:16],
                in_=sqpart[:].rearrange("p g grp -> p grp g"),
                axis=mybir.AxisListType.X,
            )
            ones = singles.tile([128, 1], F32)
            nc.vector.memset(ones[:], 1.0)
            st_ps = ps_y.tile([16, 1], F32, tag="yr_ps")
            nc.tensor.matmul(
                out=st_ps[:], lhsT=sums16[:], rhs=ones[:], start=True, stop=True
            )
            st_sb = statp.tile([16, 1], F32)
            nc.vector.tensor_copy(out=st_sb[:], in_=st_ps[:])
            bounce_in = dram.tile([16, 1], F32)
            bounce_out = dram.tile([16, 1], F32)
            nc.sync.dma_start(out=bounce_in[:], in_=st_sb[:])
            nc.gpsimd.collective_compute(
                "AllReduce",
                mybir.AluOpType.add,
                replica_groups=[list(range(cfg.NCORES))],
                ins=[bounce_in.opt()],
                outs=[bounce_out.opt()],
            )
            st16 = statp.tile([1, 16], F32)
            nc.sync.dma_start(out=st16[:], in_=bounce_out[:].rearrange("a b -> b a"))

            # mean/var -> per-channel affine A, B
            inv_cnt = 1.0 / (float(n_total_points) * cg)
            mean8 = statp.tile([1, GROUPS], F32)
            nc.vector.tensor_scalar_mul(mean8[:], st16[:, 0:GROUPS], inv_cnt)
            msq8 = statp.tile([1, GROUPS], F32)
            nc.vector.tensor_scalar_mul(msq8[:], st16[:, GROUPS:16], inv_cnt)
            var8 = statp.tile([1, GROUPS], F32)
            nc.vector.tensor_tensor(
                out=var8[:], in0=mean8[:], in1=mean8[:], op=mybir.AluOpType.mult
            )
            nc.vector.tensor_tensor(
                out=var8[:], in0=msq8[:], in1=var8[:], op=mybir.AluOpType.subtract
            )
            eps_t = statp.tile([1, 1], F32)
            nc.vector.memset(eps_t[:], EPS)
            sd8 = statp.tile([1, GROUPS], F32)
            nc.scalar.activation(
                out=sd8[:],
                in_=var8[:],
                func=mybir.ActivationFunctionType.Sqrt,
                bias=eps_t[:],
                scale=1.0,
            )
            rstd8 = statp.tile([1, GROUPS], F32)
            nc.vector.reciprocal(out=rstd8[:], in_=sd8[:])

            gam = statp.tile([1, C_OUT], F32)
            bet = statp.tile([1, C_OUT], F32)
            nc.sync.dma_start(out=gam[:], in_=gamma[:, :])
            nc.sync.dma_start(out=bet[:], in_=beta[:, :])
            rstd64 = statp.tile([1, GROUPS, cg], F32)
            nc.vector.tensor_copy(
                out=rstd64[:],
                in_=rstd8[:].rearrange("p g -> p g ()").to_broadcast([1, GROUPS, cg]),
            )
            mean64 = statp.tile([1, GROUPS, cg], F32)
            nc.vector.tensor_copy(
                out=mean64[:],
                in_=mean8[:].rearrange("p g -> p g ()").to_broadcast([1, GROUPS, cg]),
            )
            a1 = statp.tile([1, C_OUT], F32)
            nc.vector.tensor_tensor(
                out=a1[:],
                in0=rstd64[:].rearrange("p g c -> p (g c)"),
                in1=gam[:],
                op=mybir.AluOpType.mult,
            )
            b1 = statp.tile([1, C_OUT], F32)
            nc.vector.tensor_tensor(
                out=b1[:],
                in0=mean64[:].rearrange("p g c -> p (g c)"),
                in1=a1[:],
                op=mybir.AluOpType.mult,
            )
            nc.vector.tensor_tensor(
                out=b1[:], in0=bet[:], in1=b1[:], op=mybir.AluOpType.subtract
            )
            ab_dram = dram.tile([2, C_OUT], F32)
            nc.sync.dma_start(out=ab_dram[0:1, :], in_=a1[:])
            nc.sync.dma_start(out=ab_dram[1:2, :], in_=b1[:])
            a128 = singles.tile([128, C_OUT], F32)
            b128 = singles.tile([128, C_OUT], F32)
            nc.sync.dma_start(out=a128[:], in_=ab_dram[0:1, :].partition_broadcast(128))
            nc.sync.dma_start(out=b128[:], in_=ab_dram[1:2, :].partition_broadcast(128))

            # ---------------- Phase 4: normalize + LeakyReLU + store ----------
            for g in range(G):
                blk = acc[:, g * J : (g + 1) * J, :]
                t0 = nrmp.tile([128, J, C_OUT], F32)
                nc.vector.tensor_tensor(
                    out=t0[:],
                    in0=blk,
                    in1=a128[:].rearrange("p c -> p () c").to_broadcast(
                        [128, J, C_OUT]
                    ),
                    op=mybir.AluOpType.mult,
                )
                nc.vector.tensor_tensor(
                    out=t0[:],
                    in0=t0[:],
                    in1=b128[:].rearrange("p c -> p () c").to_broadcast(
                        [128, J, C_OUT]
                    ),
                    op=mybir.AluOpType.add,
                )
                t1 = nrmp.tile([128, J, C_OUT], F32)
                nc.vector.tensor_scalar_mul(t1[:], t0[:], NEG_SLOPE)
                nc.vector.tensor_tensor(
                    out=t0[:], in0=t0[:], in1=t1[:], op=mybir.AluOpType.max
                )
                nc.sync.dma_start(
                    out=outt[g * J * 128 : (g + 1) * J * 128, :].rearrange(
                        "(t p) c -> p t c", p=128
                    ),
                    in_=t0[:],
                )

    nc.compile()
    return nc


def _run(cfg, inputs, trace=False):
    from concourse import bass_utils

    in_maps, meta = host_prep(
        cfg,
        inputs["feats"],
        inputs["weight"],
        inputs["gamma"],
        inputs["beta"],
        inputs["in_idx"],
        inputs["out_idx"],
    )
    nc = build_program(cfg, meta["sched"], cfg.N)
    res = bass_utils.run_bass_kernel_spmd(
        nc, in_maps, core_ids=list(range(cfg.NCORES)), trace=trace
    )
    out = np.zeros((cfg.N, C_OUT), dtype=np.float32)
    for c in range(cfg.NCORES):
        oc = res.results[c]["out"]
        order = meta["order_rows"][c]
        out[c * cfg.R + order] = oc[: cfg.R]
    return out, res


def kernel(**inputs) -> np.ndarray:
    cfg = Cfg(N_POINTS, N_PAIRS, K_OFFSETS, NCORES)
    out, _ = _run(cfg, inputs, trace=False)
    return out



# revision 22
# speedup vs baseline: 1.1817x; 1.0033x over previous
"""Trainium2 Bass kernel for nn_BasicConvolutionBlock (sparse conv rulebook +
GroupNorm + LeakyReLU), sharded over 8 NeuronCores.

Strategy
--------
- Shard the 300000 output rows across 8 cores (37500 rows each). Every pair
  (k, p) is owned by the core that owns out_idx[k, p]. Weights replicated.
- Per core, pairs are processed grouped by kernel offset k:
    gather feats rows (indirect DMA) -> PE transpose -> matmul with W[k]
    (4-way row-group packing) -> PE transpose back -> Y rows -> DRAM scratch.
- The scatter-add is performed as an indirect *gather-accumulate*: on the host
  we sort each core's rows by pair count, bucket them into 128-row tiles, and
  build per-(tile-group, slot) index tables addressing the Y scratch. A DMA
  gather with compute_op=add accumulates slot l of every row directly into an
  SBUF accumulator. Padding slots use an out-of-bounds index which the DMA
  bounds check skips.
- GroupNorm stats: free-dim reduces + a ones-vector matmul for the partition
  reduction, then a 16-float AllReduce across the 8 cores; affine + LeakyReLU
  fused afterwards; host inverts the row permutation.
"""

import os
import sys

import ml_dtypes
import numpy as np

sys.path.insert(0, "/opt/trn_rl_repo")

import concourse.bacc as bacc
import concourse.bass as bass
import concourse.tile as tile
from concourse import mybir
from concourse.masks import make_identity

F32 = mybir.dt.float32
BF16 = mybir.dt.bfloat16
I32 = mybir.dt.int32

# Full-size problem config (hardcoded; see spec)
N_POINTS = 300000
N_PAIRS = 100000
K_OFFSETS = 27
C_IN = 32
C_OUT = 64
GROUPS = 8
EPS = 1e-5
NEG_SLOPE = 0.01
NCORES = 8


class Cfg:
    def __init__(self, n_points, n_pairs, k_offsets, ncores, xgi=None):
        self.N = n_points
        self.NPAIRS = n_pairs
        self.K = k_offsets
        self.NCORES = ncores
        self.R = n_points // ncores          # real rows per core
        self.J = 16                          # chunks per gather instruction
        self.CHUNK = 128 * self.J            # pairs per gather instruction
        self.T = -(-self.R // 128)           # tiles of 128 rows (ceil)
        self.G = -(-self.T // self.J)        # tile groups of J tiles
        self.RT = self.G * self.J * 128      # padded rows per core
        if xgi is None:
            # instructions per k-offset: max pairs per (core, k) + safety
            mean = n_pairs / ncores
            xgi = int(np.ceil((mean + 6.0 * np.sqrt(mean) + 64) / self.CHUNK))
        self.XGI = xgi
        self.NY = self.K * self.XGI * self.CHUNK   # Y scratch rows per core


def host_prep(cfg, feats, weight, gamma, beta, in_idx, out_idx):
    """Build per-core input arrays + global slot schedule."""
    K, R, CHUNK, XGI, J = cfg.K, cfg.R, cfg.CHUNK, cfg.XGI, cfg.J
    n = cfg.N

    ii = np.ascontiguousarray(in_idx, dtype=np.int64).ravel()
    oo = np.ascontiguousarray(out_idx, dtype=np.int64).ravel()
    kk = np.repeat(np.arange(K, dtype=np.int32), cfg.NPAIRS)

    feats_aug = np.zeros((n + 1, C_IN), dtype=np.float32)
    feats_aug[:n] = np.asarray(feats, dtype=np.float32)

    owner = oo // R

    per_core = []
    L_group = np.zeros((cfg.NCORES, cfg.G), dtype=np.int64)
    for c in range(cfg.NCORES):
        sel = np.nonzero(owner == c)[0]
        kk_c = kk[sel]
        ii_c = ii[sel].astype(np.int64)
        rr_c = (oo[sel] - c * R).astype(np.int64)

        # ---- k-grouping and Y positions ----
        ordk = np.argsort(kk_c, kind="stable")
        nk = np.bincount(kk_c, minlength=K)
        assert nk.max() <= XGI * CHUNK, f"XGI too small: {nk.max()}"
        offs = np.zeros(K + 1, dtype=np.int64)
        np.cumsum(nk, out=offs[1:])
        pos_in_k = np.arange(len(ordk), dtype=np.int64) - offs[kk_c[ordk]]
        ypos = np.empty(len(ordk), dtype=np.int64)
        ypos[ordk] = kk_c[ordk] * (XGI * CHUNK) + pos_in_k

        # xg_idx[k, i, p, j] = in_idx of pair with ypos k*XGI*CHUNK + i*CHUNK + j*128 + p
        xg = np.full((K, XGI * CHUNK), n, dtype=np.int64)  # dummy -> zero row
        ii_sorted = ii_c[ordk]
        for k in range(K):
            xg[k, : nk[k]] = ii_sorted[offs[k] : offs[k + 1]]
        xg_idx = (
            xg.reshape(K, XGI, J, 128).transpose(0, 1, 3, 2).astype(np.int32)
        )  # [K, XGI, 128, J]
        # pre-gathered features in processing order: [K*XGI, 128, J*C_IN]
        feats_seq = feats_aug[xg_idx.reshape(K * XGI, 128, J)].reshape(
            K * XGI, 128, J * C_IN
        ).astype(ml_dtypes.bfloat16)

        # ---- row ranking by pair count (slot padding efficiency) ----
        counts = np.bincount(rr_c, minlength=R)
        order_rows = np.argsort(-counts, kind="stable")       # rank -> orig row
        rank_of_row = np.empty(R, dtype=np.int64)
        rank_of_row[order_rows] = np.arange(R)
        counts_sorted = counts[order_rows]

        # per-tile-group max L
        for g in range(cfg.G):
            s = g * J * 128
            L_group[c, g] = counts_sorted[s] if s < R else 0

        # ---- slot assignment ----
        ranks = rank_of_row[rr_c]
        ords = np.argsort(ranks, kind="stable")
        csort = np.zeros(R + 1, dtype=np.int64)
        np.cumsum(counts_sorted, out=csort[1:])
        slot = np.arange(len(ords), dtype=np.int64) - csort[ranks[ords]]
        per_core.append(
            dict(
                xg_idx=xg_idx,
                feats_seq=feats_seq,
                rank=ranks[ords],
                slot=slot,
                ypos=ypos[ords],
                order_rows=order_rows,
            )
        )

    # ---- global slot schedule ----
    Lg = L_group.max(axis=0)  # [G]
    maxL = int(Lg.max()) if len(Lg) else 0
    sched = [(g, l) for l in range(maxL) for g in range(cfg.G) if l < Lg[g]]
    S = len(sched)
    sched_pos = np.full((cfg.G, maxL if maxL else 1), -1, dtype=np.int64)
    for s, (g, l) in enumerate(sched):
        sched_pos[g, l] = s

    in_maps = []
    for c in range(cfg.NCORES):
        pc = per_core[c]
        slot_idx = np.full((S, 128, J), cfg.NY, dtype=np.int64)  # NY = invalid
        rank, slot, ypos = pc["rank"], pc["slot"], pc["ypos"]
        g_arr = rank // (J * 128)
        t_arr = (rank % (J * 128)) // 128
        p_arr = rank % 128
        s_arr = sched_pos[g_arr, slot]
        assert (s_arr >= 0).all()
        slot_idx[s_arr, p_arr, t_arr] = ypos
        nch = K * XGI
        nca = nch // 2
        in_maps.append(
            {
                "feats_seq_a": pc["feats_seq"][:nca],
                "feats_seq_b": pc["feats_seq"][nca:],
                "wmat": np.asarray(weight, dtype=np.float32).astype(
                    ml_dtypes.bfloat16
                ),
                "gamma": np.asarray(gamma, dtype=np.float32).reshape(1, C_OUT),
                "beta": np.asarray(beta, dtype=np.float32).reshape(1, C_OUT),
                "slot_idx": slot_idx.astype(np.int32),
            }
        )

    meta = dict(sched=sched, order_rows=[pc["order_rows"] for pc in per_core])
    return in_maps, meta


def build_program(cfg, sched, n_total_points):
    K, XGI, J, CHUNK, G = cfg.K, cfg.XGI, cfg.J, cfg.CHUNK, cfg.G
    S = len(sched)
    NY = cfg.NY

    nc = bacc.Bacc(
        "TRN2", target_bir_lowering=False, debug=False, num_devices=cfg.NCORES
    )

    NCH = K * XGI
    NCA = NCH // 2
    fs_a = nc.dram_tensor(
        "feats_seq_a", [NCA, 128, J * C_IN], BF16, kind="ExternalInput"
    )
    fs_b = nc.dram_tensor(
        "feats_seq_b", [NCH - NCA, 128, J * C_IN], BF16, kind="ExternalInput"
    )
    wmat = nc.dram_tensor("wmat", [K, C_IN, C_OUT], BF16, kind="ExternalInput")
    gamma = nc.dram_tensor("gamma", [1, C_OUT], F32, kind="ExternalInput")
    beta = nc.dram_tensor("beta", [1, C_OUT], F32, kind="ExternalInput")
    sl = nc.dram_tensor("slot_idx", [S, 128, J], I32, kind="ExternalInput")
    outt = nc.dram_tensor("out", [cfg.RT, C_OUT], F32, kind="ExternalOutput")
    y_scr = nc.dram_tensor("y_scratch", [NY + 1, C_OUT], F32)

    with tile.TileContext(nc) as tc:
        with (
            tc.tile_pool(name="singles", bufs=1) as singles,
            tc.tile_pool(name="idxp", bufs=4) as idxp,
            tc.tile_pool(name="stgp", bufs=6) as stgp,
            tc.tile_pool(name="xtp", bufs=6) as xtp,
            tc.tile_pool(name="ytp", bufs=6) as ytp,
            tc.tile_pool(name="ysbp", bufs=6) as ysbp,
            tc.tile_pool(name="nrmp", bufs=3) as nrmp,
            tc.tile_pool(name="statp", bufs=1) as statp,
            tc.tile_pool(name="ps_t", bufs=2, space="PSUM") as ps_t,
            tc.tile_pool(name="ps_mm", bufs=1, space="PSUM") as ps_mm,
            tc.tile_pool(name="ps_y", bufs=2, space="PSUM") as ps_y,
            tc.tile_pool(name="dram", bufs=1, space="DRAM") as dram,
        ):
            ident = singles.tile([128, 128], F32)
            make_identity(nc, ident[:])
            identb = singles.tile([128, 128], BF16)
            make_identity(nc, identb[:])
            w4 = singles.tile([128, K, C_OUT], BF16)
            wsrc = wmat[:, :, :].rearrange("k i o -> i k o")
            for jj in range(4):
                nc.sync.dma_start(
                    out=w4[32 * jj : 32 * jj + 32, :, :],
                    in_=wsrc,
                )
            acc = singles.tile([128, G * J, C_OUT], F32)
            nc.vector.memset(acc[:], 0.0)
            zrow = singles.tile([1, C_OUT], F32)
            nc.vector.memset(zrow[:], 0.0)
            nc.sync.dma_start(out=y_scr[NY : NY + 1, :], in_=zrow[:])

            # ---------------- Phase 1: streamed feats + conv, Y to DRAM -------
            for k in range(K):
                for i in range(XGI):
                    stg = stgp.tile([128, J, C_IN], BF16)
                    ch = k * XGI + i
                    fsrc = fs_a[ch, :, :] if ch < NCA else fs_b[ch - NCA, :, :]
                    nc.sync.dma_start(
                        out=stg[:].rearrange("p j c -> p (j c)"),
                        in_=fsrc,
                    )
                    ysb = ysbp.tile([128, J, C_OUT], F32)
                    for m in range(J // 4):
                        xt_ps = ps_t.tile([128, 128], BF16)
                        nc.tensor.transpose(
                            out=xt_ps[:],
                            in_=stg[:, 4 * m : 4 * m + 4, :].rearrange(
                                "p a b -> p (a b)"
                            ),
                            identity=identb[:],
                        )
                        xt = xtp.tile([128, 128], BF16)
                        nc.vector.tensor_copy(out=xt[:], in_=xt_ps[:])
                        yt_ps = ps_mm.tile([64, 4, 512], F32)
                        for jj in range(4):
                            nc.tensor.matmul(
                                out=yt_ps[:, jj, :128],
                                lhsT=w4[32 * jj : 32 * jj + 32, k, :],
                                rhs=xt[32 * jj : 32 * jj + 32, :],
                                start=True,
                                stop=True,
                                tile_position=(32 * jj, 0),
                            )
                        yt = ytp.tile([64, 4, 128], F32)
                        nc.vector.tensor_copy(out=yt[:], in_=yt_ps[:, :, :128])
                        for jj in range(4):
                            yr_ps = ps_y.tile([128, 64], F32)
                            nc.tensor.transpose(
                                out=yr_ps[:],
                                in_=yt[:, jj, :],
                                identity=ident[:64, :64],
                            )
                            nc.scalar.copy(
                                out=ysb[:, 4 * m + jj, :], in_=yr_ps[:]
                            )
                    base = (k * XGI + i) * CHUNK
                    nc.sync.dma_start(
                        out=y_scr[base : base + CHUNK, :].rearrange(
                            "(j p) c -> p j c", p=128
                        ),
                        in_=ysb[:],
                    )

            # ---------------- Phase 2: gather + DVE-accumulate into acc -------
            for s, (g, l) in enumerate(sched):
                it = idxp.tile([128, J], I32)
                nc.sync.dma_start(out=it[:], in_=sl[s, :, :])
                stg2 = ysbp.tile([128, J, C_OUT], F32, tag="stg2")
                for tx in range(J):
                    nc.gpsimd.indirect_dma_start(
                        out=stg2[:, tx, :],
                        out_offset=None,
                        in_=y_scr[:, :],
                        in_offset=bass.IndirectOffsetOnAxis(
                            ap=it[:, tx : tx + 1], axis=0
                        ),
                    )
                nc.vector.tensor_tensor(
                    out=acc[:, g * J : (g + 1) * J, :],
                    in0=acc[:, g * J : (g + 1) * J, :],
                    in1=stg2[:],
                    op=mybir.AluOpType.add,
                )

            # ---------------- Phase 3: GroupNorm stats + AllReduce ------------
            cg = C_OUT // GROUPS
            sums16 = statp.tile([128, 16], F32)
            accv = acc[:].rearrange("p t (grp c) -> p grp t c", grp=GROUPS, c=cg)
            nc.vector.reduce_sum(
                out=sums16[:, 0:GROUPS], in_=accv, axis=mybir.AxisListType.XY
            )
            sqpart = statp.tile([128, G, GROUPS], F32)
            for g in range(G):
                sq = nrmp.tile([128, J * C_OUT], F32)
                blk = acc[:, g * J : (g + 1) * J, :].rearrange("p t c -> p (t c)")
                nc.vector.tensor_tensor(
                    out=sq[:], in0=blk, in1=blk, op=mybir.AluOpType.mult
                )
                nc.vector.reduce_sum(
                    out=sqpart[:, g, :],
                    in_=sq[:].rearrange("p (t grp c) -> p grp t c", grp=GROUPS, c=cg),
                    axis=mybir.AxisListType.XY,
                )
            nc.vector.reduce_sum(
                out=sums16[:, GROUPS:16],
                in_=sqpart[:].rearrange("p g grp -> p grp g"),
                axis=mybir.AxisListType.X,
            )
            ones = singles.tile([128, 1], F32)
            nc.vector.memset(ones[:], 1.0)
            st_ps = ps_y.tile([16, 1], F32, tag="yr_ps")
            nc.tensor.matmul(
                out=st_ps[:], lhsT=sums16[:], rhs=ones[:], start=True, stop=True
            )
            st_sb = statp.tile([16, 1], F32)
            nc.vector.tensor_copy(out=st_sb[:], in_=st_ps[:])
            bounce_in = dram.tile([16, 1], F32)
            bounce_out = dram.tile([16, 1], F32)
            nc.sync.dma_start(out=bounce_in[:], in_=st_sb[:])
            nc.gpsimd.collective_compute(
                "AllReduce",
                mybir.AluOpType.add,
                replica_groups=[list(range(cfg.NCORES))],
                ins=[bounce_in.opt()],
                outs=[bounce_out.opt()],
            )
            st16 = statp.tile([1, 16], F32)
            nc.sync.dma_start(out=st16[:], in_=bounce_out[:].rearrange("a b -> b a"))

            # mean/var -> per-channel affine A, B
            inv_cnt = 1.0 / (float(n_total_points) * cg)
            mean8 = statp.tile([1, GROUPS], F32)
            nc.vector.tensor_scalar_mul(mean8[:], st16[:, 0:GROUPS], inv_cnt)
            msq8 = statp.tile([1, GROUPS], F32)
            nc.vector.tensor_scalar_mul(msq8[:], st16[:, GROUPS:16], inv_cnt)
            var8 = statp.tile([1, GROUPS], F32)
            nc.vector.tensor_tensor(
                out=var8[:], in0=mean8[:], in1=mean8[:], op=mybir.AluOpType.mult
            )
            nc.vector.tensor_tensor(
                out=var8[:], in0=msq8[:], in1=var8[:], op=mybir.AluOpType.subtract
            )
            eps_t = statp.tile([1, 1], F32)
            nc.vector.memset(eps_t[:], EPS)
            sd8 = statp.tile([1, GROUPS], F32)
            nc.scalar.activation(
                out=sd8[:],
                in_=var8[:],
                func=mybir.ActivationFunctionType.Sqrt,
                bias=eps_t[:],
                scale=1.0,
            )
            rstd8 = statp.tile([1, GROUPS], F32)
            nc.vector.reciprocal(out=rstd8[:], in_=sd8[:])

            gam = statp.tile([1, C_OUT], F32)
            bet = statp.tile([1, C_OUT], F32)
            nc.sync.dma_start(out=gam[:], in_=gamma[:, :])
            nc.sync.dma_start(out=bet[:], in_=beta[:, :])
            rstd64 = statp.tile([1, GROUPS, cg], F32)
            nc.vector.tensor_copy(
                out=rstd64[:],
                in_=rstd8[:].rearrange("p g -> p g ()").to_broadcast([1, GROUPS, cg]),
            )
            mean64 = statp.tile([1, GROUPS, cg], F32)
            nc.vector.tensor_copy(
                out=mean64[:],
                in_=mean8[:].rearrange("p g -> p g ()").to_broadcast([1, GROUPS, cg]),
            )
            a1 = statp.tile([1, C_OUT], F32)
            nc.vector.tensor_tensor(
                out=a1[:],
                in0=rstd64[:].rearrange("p g c -> p (g c)"),
                in1=gam[:],
                op=mybir.AluOpType.mult,
            )
            b1 = statp.tile([1, C_OUT], F32)
            nc.vector.tensor_tensor(
                out=b1[:],
                in0=mean64[:].rearrange("p g c -> p (g c)"),
                in1=a1[:],
                op=mybir.AluOpType.mult,
            )
            nc.vector.tensor_tensor(
                out=b1[:], in0=bet[:], in1=b1[:], op=mybir.AluOpType.subtract
            )
            ab_dram = dram.tile([2, C_OUT], F32)
            nc.sync.dma_start(out=ab_dram[0:1, :], in_=a1[:])
            nc.sync.dma_start(out=ab_dram[1:2, :], in_=b1[:])
            a128 = singles.tile([128, C_OUT], F32)
            b128 = singles.tile([128, C_OUT], F32)
            nc.sync.dma_start(out=a128[:], in_=ab_dram[0:1, :].partition_broadcast(128))
            nc.sync.dma_start(out=b128[:], in_=ab_dram[1:2, :].partition_broadcast(128))

            # ---------------- Phase 4: normalize + LeakyReLU + store ----------
            for g in range(G):
                blk = acc[:, g * J : (g + 1) * J, :]
                t0 = nrmp.tile([128, J, C_OUT], F32)
                nc.vector.tensor_tensor(
                    out=t0[:],
                    in0=blk,
                    in1=a128[:].rearrange("p c -> p () c").to_broadcast(
                        [128, J, C_OUT]
                    ),
                    op=mybir.AluOpType.mult,
                )
                nc.vector.tensor_tensor(
                    out=t0[:],
                    in0=t0[:],
                    in1=b128[:].rearrange("p c -> p () c").to_broadcast(
                        [128, J, C_OUT]
                    ),
                    op=mybir.AluOpType.add,
                )
                t1 = nrmp.tile([128, J, C_OUT], F32)
                nc.vector.tensor_scalar_mul(t1[:], t0[:], NEG_SLOPE)
                nc.vector.tensor_tensor(
                    out=t0[:], in0=t0[:], in1=t1[:], op=mybir.AluOpType.max
                )
                nc.sync.dma_start(
                    out=outt[g * J * 128 : (g + 1) * J * 128, :].rearrange(
                        "(t p) c -> p t c", p=128
                    ),
                    in_=t0[:],
                )

    nc.compile()
    return nc


def _run(cfg, inputs, trace=False):
    from concourse import bass_utils

    in_maps, meta = host_prep(
        cfg,
        inputs["feats"],
        inputs["weight"],
        inputs["gamma"],
        inputs["beta"],
        inputs["in_idx"],
        inputs["out_idx"],
    )
    nc = build_program(cfg, meta["sched"], cfg.N)
    res = bass_utils.run_bass_kernel_spmd(
        nc, in_maps, core_ids=list(range(cfg.NCORES)), trace=trace
    )
    out = np.zeros((cfg.N, C_OUT), dtype=np.float32)
    for c in range(cfg.NCORES):
        oc = res.results[c]["out"]
        order = meta["order_rows"][c]
        out[c * cfg.R + order] = oc[: cfg.R]
    return out, res


def kernel(**inputs) -> np.ndarray:
    cfg = Cfg(N_POINTS, N_PAIRS, K_OFFSETS, NCORES)
    out, _ = _run(cfg, inputs, trace=False)
    return out

